# revision 2
# baseline (speedup 1.0000x reference)
"""Causal Grassmann Mixer — Trainium2 Bass kernel (8 NeuronCores, SPMD).

Sharding: data-parallel over B and sequence-parallel over L.
  core c -> batch b = c // 2, sequence half = c % 2 (2048 tokens each),
  plus a 32-token halo of h (the max offset) prepended on the host, so no
  cross-core communication is needed at all.

Device layout is feature-major everywhere: features on SBUF partitions,
tokens on the free dim.  The host pre-transposes h (bf16 + fp8 copies);
the per-core output comes back feature-major and is transposed back on
the host.

Math restructuring vs the reference:
  z = h@red_w (M=16 matmul), then ZI/ZJ = sel_ij @ z (K=16 one-hot
     selection matmuls) so the causal shift by d is a column offset.
  sum_d gelu(a_d) @ g2_w = (sum_d gelu(a_d)) @ g2_w : one g2 matmul.
  geom mean: count(t)=6 for t>=32; 1/6 folded into g2_w on the host, the
     first 512 tokens get an exact per-token correction vector.
  gate: logits = h @ gw1 + S @ W2p + bias_g, where W2p = (g2_w/6) @ gw2
     and bias_g = g2_b @ gw2 + gate_b are folded on the HOST (the g-part
     contracts over d_geom=256 instead of D=1024).  The h-part runs in
     fp8 DoubleRow (gw1 scaled by 64 into fp8 range; the 1/64 is folded
     into the activation scale).  sigmoid(x) = (1+tanh(x/2))/2 is fused
     into the blend, keeping every ACT op in the gelu table:
       out = 0.5*(1+t)*(h-g) + g,  t = tanh(logits/2).
"""

import numpy as np
import ml_dtypes

B, L, D = 4, 4096, 1024
R = 16
PLU = 120
DG = 256
OFFSETS = (1, 2, 4, 8, 16, 32)
HALO = 32
IDX_I, IDX_J = np.triu_indices(R, k=1)

NCORES = 8
TOK = 2048          # own tokens per core
TB = TOK + HALO     # token buffer incl. halo
T = 512             # token tile (one PSUM bank of fp32)
NT = TOK // T       # 4 output tiles per core
KD = D // 128       # 8 k-chunks of the model dim
WSCALE = 64.0       # fp8 scaling of gw1 (half its values are subnormal raw)

BF16 = ml_dtypes.bfloat16

_CACHE = {}


def _build_program(gelu_name="Gelu"):
    import concourse.bass as bass
    import concourse.mybir as mybir
    import concourse.tile as tile
    from concourse import bacc

    f32 = mybir.dt.float32
    bf16 = mybir.dt.bfloat16
    f8 = mybir.dt.float8e4
    AF = mybir.ActivationFunctionType
    ALU = mybir.AluOpType
    GELU = getattr(AF, gelu_name)
    DR = mybir.MatmulPerfMode.DoubleRow

    nc = bacc.Bacc(
        "TRN2",
        target_bir_lowering=False,
        debug=False,
        enable_asserts=False,
        num_devices=NCORES,
    )

    # ---- DRAM I/O ----
    h_t = nc.dram_tensor("h_t", [D, TB], bf16, kind="ExternalInput").ap()
    h8_d = nc.dram_tensor("h8", [D, TOK], f8, kind="ExternalInput").ap()
    redw = nc.dram_tensor("redw", [D, R], bf16, kind="ExternalInput").ap()
    sel_d = nc.dram_tensor("sel", [R, 2 * PLU], bf16, kind="ExternalInput").ap()
    rbij = nc.dram_tensor("rbij", [PLU, 2], f32, kind="ExternalInput").ap()
    g1w = nc.dram_tensor("g1w", [PLU, DG], bf16, kind="ExternalInput").ap()
    g1b = nc.dram_tensor("g1b", [128, 2], f32, kind="ExternalInput").ap()
    g2w = nc.dram_tensor("g2w", [DG, D], bf16, kind="ExternalInput").ap()
    g2b = nc.dram_tensor("g2b", [128, KD], f32, kind="ExternalInput").ap()
    gw18 = nc.dram_tensor("gw18", [D, D], f8, kind="ExternalInput").ap()
    w2p = nc.dram_tensor("w2p", [DG, D], bf16, kind="ExternalInput").ap()
    gtb2 = nc.dram_tensor("gtb2", [128, KD], f32, kind="ExternalInput").ap()
    corr = nc.dram_tensor("corr", [1, T], bf16, kind="ExternalInput").ap()
    rsel_d = nc.dram_tensor("rsel", [12, 12 * PLU], bf16, kind="ExternalInput").ap()
    out_t = nc.dram_tensor("out_t", [D, TOK], bf16, kind="ExternalOutput").ap()

    with tile.TileContext(nc) as tc:
        from contextlib import ExitStack

        ctx = ExitStack()
        with ctx:
            singles = ctx.enter_context(tc.tile_pool(name="singles", bufs=1))
            work = ctx.enter_context(tc.tile_pool(name="work", bufs=3))
            psum = ctx.enter_context(tc.tile_pool(name="psum", bufs=4, space="PSUM"))
            psul = ctx.enter_context(tc.tile_pool(name="psul", bufs=4, space="PSUM"))

            # ---- resident SBUF tensors ----
            redw_sb = singles.tile([128, KD, R], bf16)
            nc.sync.dma_start(out=redw_sb, in_=redw.rearrange("(c p) m -> p c m", p=128))
            sel_sb = singles.tile([R, 2 * PLU], bf16)
            nc.sync.dma_start(out=sel_sb, in_=sel_d)
            h_sb = singles.tile([128, KD, TB], bf16)
            h_r = h_t.rearrange("(c p) t -> p c t", p=128)
            for k in range(KD):
                nc.sync.dma_start(out=h_sb[:, k, :], in_=h_r[:, k, :])
            h8_sb = singles.tile([128, KD, TOK], f8)
            h8_r = h8_d.rearrange("(c p) t -> p c t", p=128)
            for k in range(KD):
                nc.sync.dma_start(out=h8_sb[:, k, :], in_=h8_r[:, k, :])
            gw18_sb = singles.tile([128, KD, D], f8)
            nc.sync.dma_start(out=gw18_sb, in_=gw18.rearrange("(c p) m -> p c m", p=128))
            g1w_sb = singles.tile([PLU, DG], bf16)
            nc.sync.dma_start(out=g1w_sb, in_=g1w)
            g2w_sb = singles.tile([128, 2, D], bf16)
            nc.sync.dma_start(out=g2w_sb, in_=g2w.rearrange("(c p) m -> p c m", p=128))
            w2p_sb = singles.tile([128, 2, D], bf16)
            nc.sync.dma_start(out=w2p_sb, in_=w2p.rearrange("(c p) m -> p c m", p=128))
            rbij_sb = singles.tile([PLU, 2], f32)
            nc.sync.dma_start(out=rbij_sb, in_=rbij)
            g1b_sb = singles.tile([128, 2], f32)
            nc.sync.dma_start(out=g1b_sb, in_=g1b)
            g2b_sb = singles.tile([128, KD], f32)
            nc.sync.dma_start(out=g2b_sb, in_=g2b)
            gtb2_sb = singles.tile([128, KD], f32)
            nc.sync.dma_start(out=gtb2_sb, in_=gtb2)
            corr_sb = singles.tile([1, T], bf16)
            nc.sync.dma_start(out=corr_sb, in_=corr)

            ones_m = singles.tile([1, 128], bf16)
            nc.vector.memset(ones_m, 1.0)
            # one-hot columns: onehot[:, s, m] = (m == s): the 12 (offset,
            # tile) norm reductions accumulate onto 12 distinct PSUM rows
            onehot = singles.tile([PLU, 12, 12], bf16)
            nc.vector.memset(onehot, 0.0)
            for dcol in range(12):
                nc.vector.memset(onehot[:, dcol, dcol:dcol + 1], 1.0)
            magic = singles.tile([12, T], mybir.dt.int32)
            nc.vector.memset(magic, 0x5F375A86)  # Quake rsqrt seed
            # row selector+broadcast: rsel[k, d, m] = (k == d); lhsT for the
            # K=12 matmul that broadcasts rinv row d across 120 partitions
            rsel = singles.tile([12, 12, PLU], bf16)
            nc.sync.dma_start(out=rsel, in_=rsel_d.rearrange("k (d m) -> k d m", m=PLU))

            z_sb = singles.tile([R, TB], bf16)
            zi_sb = singles.tile([PLU, TB], bf16)
            zj_sb = singles.tile([PLU, TB], bf16)
            pp_pool = ctx.enter_context(tc.tile_pool(name="pp", bufs=1))
            s_pool = ctx.enter_context(tc.tile_pool(name="spool", bufs=1))
            gfm_pool = ctx.enter_context(tc.tile_pool(name="gfmpool", bufs=1))

            # ---- phase Z: z = h @ red_w; ZI/ZJ = sel @ z (+red_b[IDX]) ----
            zchunks = [(c * T, min(T, TB - c * T)) for c in range((TB + T - 1) // T)]

            def zphase(chunks):
                for (c0, csz) in chunks:
                    zp = psum.tile([R, csz], f32, tag="ps")
                    for k in range(KD):
                        nc.tensor.matmul(
                            zp,
                            lhsT=redw_sb[:, k, :],
                            rhs=h_sb[:, k, c0:c0 + csz],
                            start=(k == 0),
                            stop=(k == KD - 1),
                        )
                    nc.scalar.copy(z_sb[:, c0:c0 + csz], zp)
                    for g, z_out in ((0, zi_sb), (1, zj_sb)):
                        sp = psum.tile([PLU, csz], f32, tag="ps")
                        nc.tensor.matmul(
                            sp,
                            lhsT=sel_sb[:, g * PLU:(g + 1) * PLU],
                            rhs=z_sb[:, c0:c0 + csz],
                            start=True,
                            stop=True,
                        )
                        nc.vector.tensor_scalar_add(
                            z_out[:, c0:c0 + csz], sp, rbij_sb[:, g:g + 1]
                        )

            out_r = out_t.rearrange("(c p) t -> p c t", p=128)
            GT = 2 * T  # two tiles per phase group
            NG = NT // 2
            sq_pool = ctx.enter_context(tc.tile_pool(name="sqp", bufs=1))
            st = {}

            def p1a(grp):
                """DVE+GpSimd: plucker p and p^2 for both tiles of the group."""
                g0 = HALO + 2 * grp * T
                pp = pp_pool.tile([PLU, 6, GT], bf16, name=f"pp{grp}", tag="pp")
                sq6 = sq_pool.tile([PLU, 6, GT], bf16, name=f"sq{grp}", tag="sq")
                st[grp] = {"pp": pp, "sq6": sq6}
                for di, delta in enumerate(OFFSETS):
                    past = slice(g0 - delta, g0 - delta + GT)
                    cur = slice(g0, g0 + GT)
                    m1 = work.tile([PLU, GT], bf16, tag="m1")
                    nc.vector.tensor_mul(m1, zi_sb[:, past], zj_sb[:, cur])
                    m2 = work.tile([PLU, GT], bf16, tag="m2")
                    nc.gpsimd.tensor_mul(m2, zj_sb[:, past], zi_sb[:, cur])
                    nc.vector.tensor_sub(pp[:, di, :], m1, m2)
                    nc.vector.tensor_mul(sq6[:, di, :], pp[:, di, :], pp[:, di, :])

            def p1b(grp):
                """Norm reduce (PE), one batched rsqrt (DVE), broadcast+scale."""
                pp, sq6 = st[grp]["pp"], st[grp]["sq6"]
                ns12 = psum.tile([12, T], f32, tag="ps", name=f"ns12_{grp}")
                for di in range(6):
                    for i in range(2):
                        nc.tensor.matmul(
                            ns12,
                            lhsT=onehot[:, 6 * i + di, :],
                            rhs=sq6[:, di, i * T:(i + 1) * T],
                            start=(di == 0 and i == 0),
                            stop=(di == 5 and i == 1),
                        )
                # rinv = rsqrt(ns + EPS^2): Quake seed + 1 Newton step
                nsf = work.tile([12, T], f32, tag="rs", bufs=4)
                nc.vector.tensor_scalar_add(nsf, ns12, 1e-12)
                sh = work.tile([12, T], mybir.dt.int32, tag="rs", bufs=4)
                nc.vector.tensor_scalar(
                    sh, nsf.bitcast(mybir.dt.int32), 1, None,
                    op0=mybir.AluOpType.arith_shift_right,
                )
                y0 = work.tile([12, T], f32, tag="rs", bufs=4)
                nc.vector.tensor_sub(y0.bitcast(mybir.dt.int32), magic, sh)
                t1 = work.tile([12, T], f32, tag="rs", bufs=4)
                nc.vector.tensor_mul(t1, y0, y0)
                nc.vector.tensor_mul(t1, t1, nsf)
                nc.vector.tensor_scalar(
                    t1, t1, -0.5, 1.5,
                    op0=mybir.AluOpType.mult, op1=mybir.AluOpType.add,
                )
                rinv = work.tile([12, T], bf16)
                nc.vector.tensor_mul(rinv, y0, t1)
                for i in range(2):
                    for di in range(6):
                        rb = psum.tile([PLU, T], f32, tag="ps")
                        nc.tensor.matmul(
                            rb, lhsT=rsel[:, 6 * i + di, :], rhs=rinv,
                            start=True, stop=True,
                        )
                        sl = slice(i * T, (i + 1) * T)
                        nc.vector.tensor_mul(pp[:, di, sl], pp[:, di, sl], rb)

            def p2part(grp):
                """a_d = p@g1_w + g1_b; S = sum_d gelu(a_d)."""
                pp = st[grp]["pp"]
                s_sb = s_pool.tile([128, 2, 2, T], bf16, name=f"s{grp}", tag="s")
                st[grp]["s"] = s_sb
                for i in range(2):
                    for di in range(6):
                        for m in range(2):
                            ap_ps = psum.tile([128, T], f32, tag="ps")
                            nc.tensor.matmul(
                                ap_ps,
                                lhsT=g1w_sb[:, m * 128:(m + 1) * 128],
                                rhs=pp[:, di, i * T:(i + 1) * T],
                                start=True,
                                stop=True,
                            )
                            if di == 0:
                                nc.scalar.activation(
                                    s_sb[:, m, i, :], ap_ps, GELU,
                                    bias=g1b_sb[:, m:m + 1],
                                )
                            else:
                                gt = work.tile([128, T], bf16)
                                nc.scalar.activation(
                                    gt, ap_ps, GELU, bias=g1b_sb[:, m:m + 1]
                                )
                                nc.vector.tensor_add(
                                    s_sb[:, m, i, :], s_sb[:, m, i, :], gt
                                )
                if grp == 0:
                    # first-tile count correction (corr==1 for t>=32)
                    corr_ps = psum.tile([128, T], f32, tag="ps")
                    nc.tensor.matmul(
                        corr_ps, lhsT=ones_m, rhs=corr_sb, start=True, stop=True
                    )
                    for m in range(2):
                        nc.vector.tensor_mul(
                            s_sb[:, m, 0, :], s_sb[:, m, 0, :], corr_ps
                        )

            def gpart(grp, which=(0, 1)):
                """G = S @ (g2_w/6) + g2_b (blend g, bf16)."""
                s_sb = st[grp]["s"]
                if "gfm" not in st[grp]:
                    st[grp]["gfm"] = gfm_pool.tile(
                        [128, KD, 2, T], bf16, name=f"gfm{grp}", tag="gfm")
                gfm_sb = st[grp]["gfm"]
                for i in which:
                    for m8 in range(KD):
                        gp = psum.tile([128, T], f32, tag="ps")
                        for k2 in range(2):
                            nc.tensor.matmul(
                                gp,
                                lhsT=g2w_sb[:, k2, m8 * 128:(m8 + 1) * 128],
                                rhs=s_sb[:, k2, i, :],
                                start=(k2 == 0),
                                stop=(k2 == 1),
                            )
                        nc.scalar.add(gfm_sb[:, m8, i, :], gp, g2b_sb[:, m8:m8 + 1])

            def bphase(grp, i):
                """gate logits (fp8 DR h-part + K=256 bf16 g-part) + blend."""
                gfm_sb = st[grp]["gfm"]
                s_sb = st[grp]["s"]
                ti = 2 * grp + i
                base = HALO + ti * T
                cur = slice(base, base + T)
                cur8 = slice(ti * T, ti * T + T)
                for m8 in range(KD):
                    lp = psul.tile([128, T], f32, tag="lp")
                    ms = slice(m8 * 128, (m8 + 1) * 128)
                    for kp in range(KD // 2):
                        nc.tensor.matmul(
                            lp,
                            lhsT=gw18_sb[:, 2 * kp:2 * kp + 2, ms],
                            rhs=h8_sb[:, 2 * kp:2 * kp + 2, cur8],
                            start=(kp == 0),
                            stop=False,
                            perf_mode=DR,
                        )
                    for k2 in range(2):
                        nc.tensor.matmul(
                            lp,
                            lhsT=w2p_sb[:, k2, ms],
                            rhs=s_sb[:, k2, i, :],
                            start=False,
                            stop=(k2 == 1),
                        )
                    # t = tanh(logits/2); logits = lp/WSCALE + bias_g
                    tt = work.tile([128, T], bf16)
                    nc.scalar.activation(
                        tt, lp, AF.Tanh,
                        bias=gtb2_sb[:, m8:m8 + 1], scale=0.5 / WSCALE,
                    )
                    # out = 0.5*(1+t)*(h-g) + g  == sigmoid(logits)*h + (1-..)*g
                    dd = work.tile([128, T], bf16)
                    nc.vector.tensor_sub(
                        dd, h_sb[:, m8, cur], gfm_sb[:, m8, i, :]
                    )
                    uu = work.tile([128, T], bf16)
                    nc.vector.scalar_tensor_tensor(
                        uu, tt, 1.0, dd, op0=ALU.add, op1=ALU.mult
                    )
                    oo = work.tile([128, T], bf16)
                    nc.vector.scalar_tensor_tensor(
                        oo, uu, 0.5, gfm_sb[:, m8, i, :],
                        op0=ALU.mult, op1=ALU.add,
                    )
                    nc.sync.dma_start(
                        out=out_r[:, m8, ti * T:(ti + 1) * T], in_=oo
                    )

            # software pipeline: P1a(g+1) before B(g) so the DVE crunches
            # the next group's plucker while the PE runs the gate; p1b(g+1)
            # between B(g)'s two tiles so its PE bits slot into gate work
            zphase(zchunks[:3])
            p1a(0)
            zphase(zchunks[3:])
            p1b(0); p2part(0); gpart(0)
            for grp in range(NG - 1):
                p1a(grp + 1)
                bphase(grp, 0)
                p1b(grp + 1)
                bphase(grp, 1)
                p2part(grp + 1)
            gpart(NG - 1, (0,))
            bphase(NG - 1, 0)
            gpart(NG - 1, (1,))
            bphase(NG - 1, 1)

    nc.compile()
    return nc


def _get_program():
    if "nc" not in _CACHE:
        _CACHE["nc"] = _build_program()
    return _CACHE["nc"]


def make_in_maps(h, red_w, red_b, g1_w, g1_b, g2_w, g2_b, gate_w, gate_b):
    """Host-side sharding + layout prep. Returns list of 8 input dicts."""
    h = np.asarray(h, np.float32)
    red_w = np.asarray(red_w, np.float32)
    red_b = np.asarray(red_b, np.float32)
    g1_w = np.asarray(g1_w, np.float32)
    g1_b = np.asarray(g1_b, np.float32)
    g2_w = np.asarray(g2_w, np.float32)
    g2_b = np.asarray(g2_b, np.float32)
    gate_w = np.asarray(gate_w, np.float32)
    gate_b = np.asarray(gate_b, np.float32)

    from concourse import mybir as _mb
    F8 = _mb.dt.np(_mb.dt.float8e4)

    redw = np.ascontiguousarray(red_w.astype(BF16))
    sel = np.zeros((R, 2 * PLU), np.float32)
    for k in range(PLU):
        sel[IDX_I[k], k] = 1.0
        sel[IDX_J[k], PLU + k] = 1.0
    sel = np.ascontiguousarray(sel.astype(BF16))
    rbij = np.ascontiguousarray(np.stack([red_b[IDX_I], red_b[IDX_J]], axis=1))
    g1w = np.ascontiguousarray(g1_w.astype(BF16))
    g1b = np.ascontiguousarray(g1_b.reshape(2, 128).T.astype(np.float32))
    g2w = np.ascontiguousarray((g2_w / 6.0).astype(BF16))
    g2b = np.ascontiguousarray(g2_b.reshape(KD, 128).T.astype(np.float32))

    gw1 = gate_w[:D]          # (D, D) h-part
    gw2 = gate_w[D:]          # (D, D) g-part
    gw18 = np.ascontiguousarray((gw1 * WSCALE).astype(F8))
    w2p_f = (g2_w / 6.0) @ gw2 * WSCALE       # (DG, D), folded g-part
    w2p = np.ascontiguousarray(w2p_f.astype(BF16))
    bias_g = g2_b @ gw2 + gate_b              # (D,)
    gtb2 = np.ascontiguousarray(
        (bias_g / 2.0).reshape(KD, 128).T.astype(np.float32))

    # per-token count correction for the first tile of a sequence
    t = np.arange(T)
    count = np.zeros(T, np.float32)
    for d in OFFSETS:
        count += (t >= d)
    corr0 = np.where(count > 0, 6.0 / np.maximum(count, 1.0), 0.0).astype(BF16)
    corr0 = corr0.reshape(1, T)
    corr1 = np.ones((1, T), BF16)

    rsel = np.zeros((12, 12, PLU), np.float32)
    for dd in range(12):
        rsel[dd, dd, :] = 1.0
    rsel = np.ascontiguousarray(rsel.reshape(12, 12 * PLU).astype(BF16))

    in_maps = []
    for c in range(NCORES):
        b, half = c // 2, c % 2
        if half == 0:
            pad = np.zeros((HALO, D), np.float32)
        else:
            pad = h[b, half * TOK - HALO: half * TOK]
        hs = np.concatenate([pad, h[b, half * TOK:(half + 1) * TOK]], axis=0)
        h_t = np.ascontiguousarray(hs.T.astype(BF16))  # (D, TB)
        h8 = np.ascontiguousarray(
            h[b, half * TOK:(half + 1) * TOK].T.astype(F8))  # (D, TOK)
        in_maps.append({
            "h_t": h_t,
            "h8": h8,
            "redw": redw,
            "sel": sel,
            "rbij": rbij,
            "g1w": g1w,
            "g1b": g1b,
            "g2w": g2w,
            "g2b": g2b,
            "gw18": gw18,
            "w2p": w2p,
            "gtb2": gtb2,
            "corr": corr0 if half == 0 else corr1,
            "rsel": rsel,
        })
    return in_maps


def assemble_output(results):
    out = np.empty((B, L, D), np.float32)
    for c in range(NCORES):
        b, half = c // 2, c % 2
        ot = np.asarray(results[c]["out_t"]).astype(np.float32)  # (D, TOK)
        out[b, half * TOK:(half + 1) * TOK, :] = ot.T
    return out


def kernel(**inputs):
    from concourse.bass_utils import run_bass_kernel_spmd

    nc = _get_program()
    in_maps = make_in_maps(**inputs)
    res = run_bass_kernel_spmd(nc, in_maps, core_ids=list(range(NCORES)))
    return assemble_output(res.results)


# revision 11
# speedup vs baseline: 1.0049x; 1.0049x over previous
"""Causal Grassmann Mixer — Trainium2 Bass kernel (8 NeuronCores, SPMD).

Sharding: data-parallel over B and sequence-parallel over L.
  core c -> batch b = c // 2, sequence half = c % 2 (2048 tokens each),
  plus a 32-token halo of h (the max offset) prepended on the host, so no
  cross-core communication is needed at all.

Device layout is feature-major everywhere: features on SBUF partitions,
tokens on the free dim.  The host pre-transposes h (bf16 + fp8 copies);
the per-core output comes back feature-major and is transposed back on
the host.

Math restructuring vs the reference:
  z = h@red_w (M=16 matmul), then ZI/ZJ = sel_ij @ z (K=16 one-hot
     selection matmuls) so the causal shift by d is a column offset.
  sum_d gelu(a_d) @ g2_w = (sum_d gelu(a_d)) @ g2_w : one g2 matmul.
  geom mean: count(t)=6 for t>=32; 1/6 folded into g2_w on the host, the
     first 512 tokens get an exact per-token correction vector.
  gate: logits = h @ gw1 + S @ W2p + bias_g, where W2p = (g2_w/6) @ gw2
     and bias_g = g2_b @ gw2 + gate_b are folded on the HOST (the g-part
     contracts over d_geom=256 instead of D=1024).  The h-part runs in
     fp8 DoubleRow (gw1 scaled by 64 into fp8 range; the 1/64 is folded
     into the activation scale).  sigmoid(x) = (1+tanh(x/2))/2 is fused
     into the blend, keeping every ACT op in the gelu table:
       out = 0.5*(1+t)*(h-g) + g,  t = tanh(logits/2).
"""

import numpy as np
import ml_dtypes

B, L, D = 4, 4096, 1024
R = 16
PLU = 120
DG = 256
OFFSETS = (1, 2, 4, 8, 16, 32)
HALO = 32
IDX_I, IDX_J = np.triu_indices(R, k=1)

NCORES = 8
TOK = 2048          # own tokens per core
TB = TOK + HALO     # token buffer incl. halo
T = 512             # token tile (one PSUM bank of fp32)
NT = TOK // T       # 4 output tiles per core
KD = D // 128       # 8 k-chunks of the model dim
WSCALE = 64.0       # fp8 scaling of gw1 (half its values are subnormal raw)

BF16 = ml_dtypes.bfloat16

_CACHE = {}


def _build_program(gelu_name="Gelu"):
    import concourse.bass as bass
    import concourse.mybir as mybir
    import concourse.tile as tile
    from concourse import bacc

    f32 = mybir.dt.float32
    bf16 = mybir.dt.bfloat16
    f8 = mybir.dt.float8e4
    AF = mybir.ActivationFunctionType
    ALU = mybir.AluOpType
    GELU = getattr(AF, gelu_name)
    DR = mybir.MatmulPerfMode.DoubleRow

    nc = bacc.Bacc(
        "TRN2",
        target_bir_lowering=False,
        debug=False,
        enable_asserts=False,
        num_devices=NCORES,
    )

    # ---- DRAM I/O ----
    h_t = nc.dram_tensor("h_t", [D, TB], bf16, kind="ExternalInput").ap()
    h8_d = nc.dram_tensor("h8", [D, TOK], f8, kind="ExternalInput").ap()
    redw = nc.dram_tensor("redw", [D, R], bf16, kind="ExternalInput").ap()
    sel_d = nc.dram_tensor("sel", [R, 2 * PLU], bf16, kind="ExternalInput").ap()
    rbij = nc.dram_tensor("rbij", [PLU, 2], f32, kind="ExternalInput").ap()
    g1w = nc.dram_tensor("g1w", [PLU, DG], bf16, kind="ExternalInput").ap()
    g1b = nc.dram_tensor("g1b", [128, 2], f32, kind="ExternalInput").ap()
    g2w = nc.dram_tensor("g2w", [DG, D], bf16, kind="ExternalInput").ap()
    g2b = nc.dram_tensor("g2b", [128, KD], f32, kind="ExternalInput").ap()
    gw18 = nc.dram_tensor("gw18", [D, D], f8, kind="ExternalInput").ap()
    w2p = nc.dram_tensor("w2p", [DG, D], bf16, kind="ExternalInput").ap()
    gtb2 = nc.dram_tensor("gtb2", [128, KD], f32, kind="ExternalInput").ap()
    corr = nc.dram_tensor("corr", [1, T], bf16, kind="ExternalInput").ap()
    rsel_d = nc.dram_tensor("rsel", [12, 12 * PLU], bf16, kind="ExternalInput").ap()
    out_t = nc.dram_tensor("out_t", [D, TOK], bf16, kind="ExternalOutput").ap()

    with tile.TileContext(nc) as tc:
        from contextlib import ExitStack

        ctx = ExitStack()
        with ctx:
            singles = ctx.enter_context(tc.tile_pool(name="singles", bufs=1))
            work = ctx.enter_context(tc.tile_pool(name="work", bufs=3))
            psum = ctx.enter_context(tc.tile_pool(name="psum", bufs=4, space="PSUM"))
            psul = ctx.enter_context(tc.tile_pool(name="psul", bufs=4, space="PSUM"))

            # ---- resident SBUF tensors ----
            redw_sb = singles.tile([128, KD, R], bf16)
            nc.sync.dma_start(out=redw_sb, in_=redw.rearrange("(c p) m -> p c m", p=128))
            sel_sb = singles.tile([R, 2 * PLU], bf16)
            nc.sync.dma_start(out=sel_sb, in_=sel_d)
            h_sb = singles.tile([128, KD, TB], bf16)
            h_r = h_t.rearrange("(c p) t -> p c t", p=128)
            for k in range(KD):
                nc.sync.dma_start(out=h_sb[:, k, :], in_=h_r[:, k, :])
            h8_sb = singles.tile([128, KD, TOK], f8)
            h8_r = h8_d.rearrange("(c p) t -> p c t", p=128)
            for k in range(KD):
                nc.sync.dma_start(out=h8_sb[:, k, :], in_=h8_r[:, k, :])
            gw18_sb = singles.tile([128, KD, D], f8)
            nc.sync.dma_start(out=gw18_sb, in_=gw18.rearrange("(c p) m -> p c m", p=128))
            g1w_sb = singles.tile([PLU, DG], bf16)
            nc.sync.dma_start(out=g1w_sb, in_=g1w)
            g2w_sb = singles.tile([128, 2, D], bf16)
            nc.sync.dma_start(out=g2w_sb, in_=g2w.rearrange("(c p) m -> p c m", p=128))
            w2p_sb = singles.tile([128, 2, D], bf16)
            nc.sync.dma_start(out=w2p_sb, in_=w2p.rearrange("(c p) m -> p c m", p=128))
            rbij_sb = singles.tile([PLU, 2], f32)
            nc.sync.dma_start(out=rbij_sb, in_=rbij)
            g1b_sb = singles.tile([128, 2], f32)
            nc.sync.dma_start(out=g1b_sb, in_=g1b)
            g2b_sb = singles.tile([128, KD], f32)
            nc.sync.dma_start(out=g2b_sb, in_=g2b)
            gtb2_sb = singles.tile([128, KD], f32)
            nc.sync.dma_start(out=gtb2_sb, in_=gtb2)
            corr_sb = singles.tile([1, T], bf16)
            nc.sync.dma_start(out=corr_sb, in_=corr)

            ones_m = singles.tile([1, 128], bf16)
            nc.vector.memset(ones_m, 1.0)
            # one-hot columns: onehot[:, s, m] = (m == s): the 12 (offset,
            # tile) norm reductions accumulate onto 12 distinct PSUM rows
            onehot = singles.tile([PLU, 12, 12], bf16)
            nc.vector.memset(onehot, 0.0)
            for dcol in range(12):
                nc.vector.memset(onehot[:, dcol, dcol:dcol + 1], 1.0)
            magic = singles.tile([12, T], mybir.dt.int32)
            nc.vector.memset(magic, 0x5F375A86)  # Quake rsqrt seed
            # row selector+broadcast: rsel[k, d, m] = (k == d); lhsT for the
            # K=12 matmul that broadcasts rinv row d across 120 partitions
            rsel = singles.tile([12, 12, PLU], bf16)
            nc.sync.dma_start(out=rsel, in_=rsel_d.rearrange("k (d m) -> k d m", m=PLU))

            z_sb = singles.tile([R, TB], bf16)
            zi_sb = singles.tile([PLU, TB], bf16)
            zj_sb = singles.tile([PLU, TB], bf16)
            pp_pool = ctx.enter_context(tc.tile_pool(name="pp", bufs=1))
            s_pool = ctx.enter_context(tc.tile_pool(name="spool", bufs=1))
            gfm_pool = ctx.enter_context(tc.tile_pool(name="gfmpool", bufs=1))
            dd_pool = ctx.enter_context(tc.tile_pool(name="ddp", bufs=2))
            al_pool = ctx.enter_context(tc.tile_pool(name="alp", bufs=2))

            # ---- phase Z: z = h @ red_w; ZI/ZJ = sel @ z (+red_b[IDX]) ----
            zchunks = [(c * T, min(T, TB - c * T)) for c in range((TB + T - 1) // T)]

            def zphase(chunks):
                for (c0, csz) in chunks:
                    zp = psum.tile([R, csz], f32, tag="ps")
                    for k in range(KD):
                        nc.tensor.matmul(
                            zp,
                            lhsT=redw_sb[:, k, :],
                            rhs=h_sb[:, k, c0:c0 + csz],
                            start=(k == 0),
                            stop=(k == KD - 1),
                        )
                    nc.scalar.copy(z_sb[:, c0:c0 + csz], zp)
                    for g, z_out in ((0, zi_sb), (1, zj_sb)):
                        sp = psum.tile([PLU, csz], f32, tag="ps")
                        nc.tensor.matmul(
                            sp,
                            lhsT=sel_sb[:, g * PLU:(g + 1) * PLU],
                            rhs=z_sb[:, c0:c0 + csz],
                            start=True,
                            stop=True,
                        )
                        nc.vector.tensor_scalar_add(
                            z_out[:, c0:c0 + csz], sp, rbij_sb[:, g:g + 1]
                        )

            out_r = out_t.rearrange("(c p) t -> p c t", p=128)
            GT = 2 * T  # two tiles per phase group
            NG = NT // 2
            sq_pool = ctx.enter_context(tc.tile_pool(name="sqp", bufs=1))
            st = {}

            def p1a(grp):
                """DVE+GpSimd: plucker p and p^2 for both tiles of the group.

                m2 runs on GpSimd (~2.2us/op) while the DVE does m1; the
                emission order paces DVE consumption (sub waits on m2) to the
                GpSimd production rate to avoid head-of-line stalls."""
                g0 = HALO + 2 * grp * T
                pp = pp_pool.tile([PLU, 6, GT], bf16, name=f"pp{grp}", tag="pp")
                sq6 = sq_pool.tile([PLU, 6, GT], bf16, name=f"sq{grp}", tag="sq")
                st[grp] = {"pp": pp, "sq6": sq6}
                cur = slice(g0, g0 + GT)
                m1s, m2s = [], []

                def emit_m(di):
                    delta = OFFSETS[di]
                    past = slice(g0 - delta, g0 - delta + GT)
                    m1 = work.tile([PLU, GT], bf16, tag="m1")
                    nc.vector.tensor_mul(m1, zi_sb[:, past], zj_sb[:, cur])
                    m2 = work.tile([PLU, GT], bf16, tag="m2")
                    nc.gpsimd.tensor_mul(m2, zj_sb[:, past], zi_sb[:, cur])
                    m1s.append(m1); m2s.append(m2)

                def emit_ss(di):
                    nc.vector.tensor_sub(pp[:, di, :], m1s[di], m2s[di])
                    nc.vector.tensor_mul(sq6[:, di, :], pp[:, di, :], pp[:, di, :])

                emit_m(0); emit_m(1)
                for di in range(4):
                    emit_ss(di); emit_m(di + 2)
                emit_ss(4); emit_ss(5)

            def p1b(grp):
                """Norm reduce (PE), one batched rsqrt (DVE), broadcast+scale."""
                pp, sq6 = st[grp]["pp"], st[grp]["sq6"]
                ns12 = psum.tile([12, T], f32, tag="ps", name=f"ns12_{grp}")
                for di in range(6):
                    for i in range(2):
                        nc.tensor.matmul(
                            ns12,
                            lhsT=onehot[:, 6 * i + di, :],
                            rhs=sq6[:, di, i * T:(i + 1) * T],
                            start=(di == 0 and i == 0),
                            stop=(di == 5 and i == 1),
                        )
                # rinv = rsqrt(ns + EPS^2): Quake seed + 1 Newton step
                nsf = work.tile([12, T], f32, tag="rs", bufs=4)
                nc.vector.tensor_scalar_add(nsf, ns12, 1e-12)
                sh = work.tile([12, T], mybir.dt.int32, tag="rs", bufs=4)
                nc.vector.tensor_scalar(
                    sh, nsf.bitcast(mybir.dt.int32), 1, None,
                    op0=mybir.AluOpType.arith_shift_right,
                )
                y0 = work.tile([12, T], f32, tag="rs", bufs=4)
                nc.vector.tensor_sub(y0.bitcast(mybir.dt.int32), magic, sh)
                t1 = work.tile([12, T], f32, tag="rs", bufs=4)
                nc.vector.tensor_mul(t1, y0, y0)
                nc.vector.tensor_mul(t1, t1, nsf)
                nc.vector.tensor_scalar(
                    t1, t1, -0.5, 1.5,
                    op0=mybir.AluOpType.mult, op1=mybir.AluOpType.add,
                )
                rinv = work.tile([12, T], bf16)
                nc.vector.tensor_mul(rinv, y0, t1)
                for i in range(2):
                    for di in range(6):
                        rb = psum.tile([PLU, T], f32, tag="ps")
                        nc.tensor.matmul(
                            rb, lhsT=rsel[:, 6 * i + di, :], rhs=rinv,
                            start=True, stop=True,
                        )
                        sl = slice(i * T, (i + 1) * T)
                        nc.vector.tensor_mul(pp[:, di, sl], pp[:, di, sl], rb)

            def p2part(grp):
                """a_d = p@g1_w + g1_b; S = sum_d gelu(a_d)."""
                pp = st[grp]["pp"]
                s_sb = s_pool.tile([128, 2, 2, T], bf16, name=f"s{grp}", tag="s")
                st[grp]["s"] = s_sb
                for i in range(2):
                    for di in range(6):
                        gt2 = None
                        for m in range(2):
                            ap_ps = psum.tile([128, T], f32, tag="ps")
                            nc.tensor.matmul(
                                ap_ps,
                                lhsT=g1w_sb[:, m * 128:(m + 1) * 128],
                                rhs=pp[:, di, i * T:(i + 1) * T],
                                start=True,
                                stop=True,
                            )
                            if di == 0:
                                nc.scalar.activation(
                                    s_sb[:, m, i, :], ap_ps, GELU,
                                    bias=g1b_sb[:, m:m + 1],
                                )
                            else:
                                if gt2 is None:
                                    gt2 = work.tile([128, 2, T], bf16, tag="gt")
                                nc.scalar.activation(
                                    gt2[:, m, :], ap_ps, GELU,
                                    bias=g1b_sb[:, m:m + 1],
                                )
                        if gt2 is not None:
                            # one batched DVE add over both dg chunks
                            nc.vector.tensor_add(
                                s_sb[:, :, i, :], s_sb[:, :, i, :], gt2
                            )
                if grp == 0:
                    # first-tile count correction (corr==1 for t>=32)
                    corr_ps = psum.tile([128, T], f32, tag="ps")
                    nc.tensor.matmul(
                        corr_ps, lhsT=ones_m, rhs=corr_sb, start=True, stop=True
                    )
                    for m in range(2):
                        nc.vector.tensor_mul(
                            s_sb[:, m, 0, :], s_sb[:, m, 0, :], corr_ps
                        )

            def gpart(grp, which=(0, 1)):
                """G = S @ (g2_w/6) + g2_b (blend g, bf16)."""
                s_sb = st[grp]["s"]
                if "gfm" not in st[grp]:
                    st[grp]["gfm"] = gfm_pool.tile(
                        [128, KD, 2, T], bf16, name=f"gfm{grp}", tag="gfm")
                gfm_sb = st[grp]["gfm"]
                for m8 in range(KD):
                    gps = {
                        i: psum.tile([128, T], f32, tag="ps", name=f"gp{grp}_{m8}_{i}")
                        for i in which
                    }
                    for k2 in range(2):
                        # same stationary weight for both halves: load once
                        for i in which:
                            nc.tensor.matmul(
                                gps[i],
                                lhsT=g2w_sb[:, k2, m8 * 128:(m8 + 1) * 128],
                                rhs=s_sb[:, k2, i, :],
                                start=(k2 == 0),
                                stop=(k2 == 1),
                            )
                    for i in which:
                        # balance PSUM->SBUF moves across ACT and DVE
                        if m8 % 2 == 0:
                            nc.scalar.add(
                                gfm_sb[:, m8, i, :], gps[i], g2b_sb[:, m8:m8 + 1])
                        else:
                            nc.vector.tensor_scalar_add(
                                gfm_sb[:, m8, i, :], gps[i], g2b_sb[:, m8:m8 + 1])

            def bphase(grp, i):
                """gate logits (fp8 DR h-part + K=256 bf16 g-part) + blend.

                alpha for all 8 m8-chunks lands in one SBUF tile, then the
                blend runs as 3 batched DVE ops over [128, KD*T] (the ~150ns
                per-op DVE overhead is paid 3x instead of 24x) and one DMA."""
                gfm_sb = st[grp]["gfm"]
                s_sb = st[grp]["s"]
                ti = 2 * grp + i
                base = HALO + ti * T
                cur = slice(base, base + T)
                cur8 = slice(ti * T, ti * T + T)
                # dd = h - g for the whole tile: ready as soon as gfm is
                dd = dd_pool.tile([128, KD, T], bf16, tag="dd")
                nc.vector.tensor_sub(dd, h_sb[:, :, cur], gfm_sb[:, :, i, :])
                alpha = al_pool.tile([128, KD, T], bf16, tag="al")
                for m8 in range(KD):
                    lp = psul.tile([128, T], f32, tag="lp")
                    ms = slice(m8 * 128, (m8 + 1) * 128)
                    for kp in range(KD // 2):
                        nc.tensor.matmul(
                            lp,
                            lhsT=gw18_sb[:, 2 * kp:2 * kp + 2, ms],
                            rhs=h8_sb[:, 2 * kp:2 * kp + 2, cur8],
                            start=(kp == 0),
                            stop=False,
                            perf_mode=DR,
                        )
                    for k2 in range(2):
                        nc.tensor.matmul(
                            lp,
                            lhsT=w2p_sb[:, k2, ms],
                            rhs=s_sb[:, k2, i, :],
                            start=False,
                            stop=(k2 == 1),
                        )
                    # alpha = sigmoid(lp/WSCALE + bias_g)
                    nc.scalar.activation(
                        alpha[:, m8, :], lp, AF.Sigmoid,
                        bias=gtb2_sb[:, m8:m8 + 1], scale=1.0 / WSCALE,
                    )
                # out = g + alpha*(h-g): two batched ops, alpha reused as out
                nc.vector.tensor_mul(dd, dd, alpha)
                nc.vector.tensor_add(alpha, gfm_sb[:, :, i, :], dd)
                nc.sync.dma_start(
                    out=out_r[:, :, ti * T:(ti + 1) * T], in_=alpha
                )

            # software pipeline: P1a(g+1) before B(g) so the DVE crunches
            # the next group's plucker while the PE runs the gate; p1b(g+1)
            # between B(g)'s two tiles so its PE bits slot into gate work
            zphase(zchunks[:3])
            p1a(0)
            zphase(zchunks[3:])
            p1b(0); p2part(0); gpart(0)
            for grp in range(NG - 1):
                p1a(grp + 1)
                bphase(grp, 0)
                p1b(grp + 1)
                bphase(grp, 1)
                p2part(grp + 1)
            gpart(NG - 1, (0,))
            bphase(NG - 1, 0)
            gpart(NG - 1, (1,))
            bphase(NG - 1, 1)

    nc.compile()
    return nc


def _get_program():
    if "nc" not in _CACHE:
        _CACHE["nc"] = _build_program()
    return _CACHE["nc"]


def make_in_maps(h, red_w, red_b, g1_w, g1_b, g2_w, g2_b, gate_w, gate_b):
    """Host-side sharding + layout prep. Returns list of 8 input dicts."""
    h = np.asarray(h, np.float32)
    red_w = np.asarray(red_w, np.float32)
    red_b = np.asarray(red_b, np.float32)
    g1_w = np.asarray(g1_w, np.float32)
    g1_b = np.asarray(g1_b, np.float32)
    g2_w = np.asarray(g2_w, np.float32)
    g2_b = np.asarray(g2_b, np.float32)
    gate_w = np.asarray(gate_w, np.float32)
    gate_b = np.asarray(gate_b, np.float32)

    from concourse import mybir as _mb
    F8 = _mb.dt.np(_mb.dt.float8e4)

    redw = np.ascontiguousarray(red_w.astype(BF16))
    sel = np.zeros((R, 2 * PLU), np.float32)
    for k in range(PLU):
        sel[IDX_I[k], k] = 1.0
        sel[IDX_J[k], PLU + k] = 1.0
    sel = np.ascontiguousarray(sel.astype(BF16))
    rbij = np.ascontiguousarray(np.stack([red_b[IDX_I], red_b[IDX_J]], axis=1))
    g1w = np.ascontiguousarray(g1_w.astype(BF16))
    g1b = np.ascontiguousarray(g1_b.reshape(2, 128).T.astype(np.float32))
    g2w = np.ascontiguousarray((g2_w / 6.0).astype(BF16))
    g2b = np.ascontiguousarray(g2_b.reshape(KD, 128).T.astype(np.float32))

    gw1 = gate_w[:D]          # (D, D) h-part
    gw2 = gate_w[D:]          # (D, D) g-part
    gw18 = np.ascontiguousarray((gw1 * WSCALE).astype(F8))
    w2p_f = (g2_w / 6.0) @ gw2 * WSCALE       # (DG, D), folded g-part
    w2p = np.ascontiguousarray(w2p_f.astype(BF16))
    bias_g = g2_b @ gw2 + gate_b              # (D,)
    gtb2 = np.ascontiguousarray(
        bias_g.reshape(KD, 128).T.astype(np.float32))

    # per-token count correction for the first tile of a sequence
    t = np.arange(T)
    count = np.zeros(T, np.float32)
    for d in OFFSETS:
        count += (t >= d)
    corr0 = np.where(count > 0, 6.0 / np.maximum(count, 1.0), 0.0).astype(BF16)
    corr0 = corr0.reshape(1, T)
    corr1 = np.ones((1, T), BF16)

    rsel = np.zeros((12, 12, PLU), np.float32)
    for dd in range(12):
        rsel[dd, dd, :] = 1.0
    rsel = np.ascontiguousarray(rsel.reshape(12, 12 * PLU).astype(BF16))

    in_maps = []
    for c in range(NCORES):
        b, half = c // 2, c % 2
        if half == 0:
            pad = np.zeros((HALO, D), np.float32)
        else:
            pad = h[b, half * TOK - HALO: half * TOK]
        hs = np.concatenate([pad, h[b, half * TOK:(half + 1) * TOK]], axis=0)
        h_t = np.ascontiguousarray(hs.T.astype(BF16))  # (D, TB)
        h8 = np.ascontiguousarray(
            h[b, half * TOK:(half + 1) * TOK].T.astype(F8))  # (D, TOK)
        in_maps.append({
            "h_t": h_t,
            "h8": h8,
            "redw": redw,
            "sel": sel,
            "rbij": rbij,
            "g1w": g1w,
            "g1b": g1b,
            "g2w": g2w,
            "g2b": g2b,
            "gw18": gw18,
            "w2p": w2p,
            "gtb2": gtb2,
            "corr": corr0 if half == 0 else corr1,
            "rsel": rsel,
        })
    return in_maps


def assemble_output(results):
    out = np.empty((B, L, D), np.float32)
    for c in range(NCORES):
        b, half = c // 2, c % 2
        ot = np.asarray(results[c]["out_t"]).astype(np.float32)  # (D, TOK)
        out[b, half * TOK:(half + 1) * TOK, :] = ot.T
    return out


def kernel(**inputs):
    from concourse.bass_utils import run_bass_kernel_spmd

    nc = _get_program()
    in_maps = make_in_maps(**inputs)
    res = run_bass_kernel_spmd(nc, in_maps, core_ids=list(range(NCORES)))
    return assemble_output(res.results)
